# revision 24
# baseline (speedup 1.0000x reference)
"""4D Conv-MLP (conv3^4 -> ReLU -> conv3^4) on 8 Trainium2 NeuronCores.

Sharding: core = b*4 + j (batch b in {0,1}, H-slab j in {0..3}, 8 output rows
each). Conv1 is recomputed on a 1-row h halo (10 h rows from 12 x rows), so no
cross-core communication. One SPMD program; boundary behavior is data-driven
(host-zeroed x halos + h halo-row masks).

Winograd F(2,3) along W on both convs (1.5x fewer multiplies): the 3 kw taps
become 4 pointwise components m0..m3 evaluated at 16 stride-2 w-tiles;
y[2k] = m0+m1+m2, y[2k+1] = m1-m2-m3. T/D/H taps (kt, ku, kv) stay direct.

  - x~ (input transform) is built on host: 4 component planes [T, D, 12, 16],
    components packed in pairs on 128 partitions (xA = c0|c1, xB = c2|c3).
  - conv1: K=64 matmuls, one PSUM accumulator chain per component; adjacent
    components alternate PE row-halves via tile_position (0,0)/(64,0) so two
    matmuls co-stream (2 cols/cycle aggregate). d processed in runs of <=3
    (N <= 480 <= one PSUM bank).
  - Inverse transform + ReLU + bias on DVE/Scalar writes h_e/h_o staging;
    h~ (conv2's forward transform, 4 components) is built from them with
    shifted adds and stored for all (t, d): [128, T, 4, D, 10, 16] fp16.
  - conv2: K=128 M=64 matmuls; components alternate PE column-halves via
    tile_position (0,0)/(0,64) (co-stream, as in the direct kernel). d runs
    of <=4 (N <= 512). Inverse + bias on DVE/Scalar, stride-2 DMA to y.
All matmul operands fp16, PSUM accumulation fp32. The PE is stream-bound
(LDWEIGHTS fully pipelines), so wall ~ total stream columns / 2 streams.
"""

import numpy as np

B, C_IN, C_HID, C_OUT = 2, 64, 128, 64
T, D, H, W = 4, 16, 32, 32
NCORES, NJ = 8, 4
SH = H // NJ          # 8 out rows per slab
XH = SH + 4           # 12 x rows per slab
HR = SH + 2           # 10 h rows per slab (1-row halo each side)
KW = W // 2           # 16 winograd w-tiles
XPL = D * XH * KW     # x~ plane per t = 3072
HPL = D * HR * KW     # h~ plane per (t, comp) = 2560

# d-runs: (dlo, nd); valid ku for a run = [max(0,1-dlo), min(2, 17-dlo-nd)]
RUNS1 = [(0, 1), (1, 3), (4, 3), (15, 1), (7, 3), (13, 2), (10, 3)]
RUNS2 = [(0, 1), (15, 1), (13, 2), (1, 4), (5, 4), (9, 4)]

_cache = {}


def _t_taps(t):
    return [kt for kt in range(3) if 0 <= t + kt - 1 < T]


def _ku_valid(dlo, nd):
    return [ku for ku in range(3) if dlo + ku - 1 >= 0 and dlo + nd + ku - 2 <= 15]


def _g27(kt, ku, kv):
    return (kt * 3 + ku) * 3 + kv


def _wino_w(g):
    """F(2,3) weight transform along the last axis (len 3) -> 4 components."""
    c0 = g[..., 0]
    c1 = 0.5 * (g[..., 0] + g[..., 1] + g[..., 2])
    c2 = 0.5 * (g[..., 0] - g[..., 1] + g[..., 2])
    c3 = g[..., 2]
    return c0, c1, c2, c3


def _make_host_arrays(x, w1, b1, w2, b2):
    x = np.asarray(x, np.float32)
    XAs, XBs, AUXs = [], [], []
    for core in range(NCORES):
        b, j = divmod(core, NJ)
        h0 = SH * j
        slab = np.zeros((C_IN, T, D, XH, W + 3), np.float32)
        lo, hi = h0 - 2, h0 + 10
        slo, shi = max(lo, 0), min(hi, H)
        slab[:, :, :, slo - lo:shi - lo, 1:33] = x[b, :, :, :, slo:shi, :]
        # winograd input transform along W: tile k reads slab cols 2k..2k+3
        d0 = slab[..., 0:32:2]
        d1 = slab[..., 1:33:2]
        d2 = slab[..., 2:34:2]
        d3 = slab[..., 3:35:2]
        xt0 = d0 - d2
        xt1 = d1 + d2
        xt2 = d2 - d1
        xt3 = d1 - d3
        XAs.append(np.concatenate([xt0, xt1], 0).astype(np.float16)
                   .reshape(128, T, XPL))
        XBs.append(np.concatenate([xt2, xt3], 0).astype(np.float16)
                   .reshape(128, T, XPL))
        a = np.zeros((128, 4), np.float32)
        a[:, 0] = np.asarray(b1, np.float32)
        a[:, 1] = 0.0 if j == 0 else 1.0
        a[:, 2] = 0.0 if j == NJ - 1 else 1.0
        a[0:64, 3] = np.asarray(b2, np.float32)
        AUXs.append(a)

    w1 = np.asarray(w1, np.float32)   # [128, 64, 3,3,3,3]
    w2 = np.asarray(w2, np.float32)   # [64, 128, 3,3,3,3]
    W1A = np.zeros((128, 27, 128), np.float16)
    W1B = np.zeros((128, 27, 128), np.float16)
    W2T = np.zeros((128, 4, 27, 64), np.float16)
    for kt in range(3):
        for ku in range(3):
            for kv in range(3):
                g = _g27(kt, ku, kv)
                c1s = _wino_w(w1[:, :, kt, ku, kv, :])   # each [128out, 64in]
                W1A[0:64, g, :] = c1s[0].T
                W1A[64:128, g, :] = c1s[1].T
                W1B[0:64, g, :] = c1s[2].T
                W1B[64:128, g, :] = c1s[3].T
                c2s = _wino_w(w2[:, :, kt, ku, kv, :])   # each [64out, 128in]
                for c in range(4):
                    W2T[:, c, g, :] = c2s[c].T
    return dict(XA=XAs, XB=XBs, AUX=AUXs,
                W1A=W1A.reshape(128, 27 * 128), W1B=W1B.reshape(128, 27 * 128),
                W2=W2T.reshape(128, 4 * 27 * 64))


def _build_module():
    import concourse.tile as tile
    from concourse import bacc, mybir

    fp16 = mybir.dt.float16
    fp32 = mybir.dt.float32
    RELU = mybir.ActivationFunctionType.Relu
    IDENT = mybir.ActivationFunctionType.Identity

    nc = bacc.Bacc("TRN2", target_bir_lowering=False, debug=False, num_devices=1)
    xa_d = nc.dram_tensor("xa", [128, T, XPL], fp16, kind="ExternalInput")
    xb_d = nc.dram_tensor("xb", [128, T, XPL], fp16, kind="ExternalInput")
    w1a_d = nc.dram_tensor("w1a", [128, 27 * 128], fp16, kind="ExternalInput")
    w1b_d = nc.dram_tensor("w1b", [128, 27 * 128], fp16, kind="ExternalInput")
    w2_d = nc.dram_tensor("w2", [128, 4 * 27 * 64], fp16, kind="ExternalInput")
    # aux: col0=b1, col1=mt, col2=mb, col3[:64]=b2
    aux_d = nc.dram_tensor("aux", [128, 4], fp32, kind="ExternalInput")
    # even / odd w-planes stored separately (fp16); host interleaves + upcasts
    y_d = nc.dram_tensor("y", [64, T, 2, D * SH * KW], fp16,
                         kind="ExternalOutput")

    with tile.TileContext(nc) as tc:
        with (
            tc.tile_pool(name="xw", bufs=1) as xw,
            tc.tile_pool(name="st", bufs=2) as stp,
            tc.tile_pool(name="pp", bufs=2, space="PSUM") as pp,
        ):
            xA = xw.tile([128, T, D, XH, KW], fp16)
            xB = xw.tile([128, T, D, XH, KW], fp16)

            # minimize serial DMA-issue count before the first matmul:
            # w1a + x~ t0/t1 + w1b (+aux) unblock the whole conv1 t=0 sweep;
            # transfers on distinct queues run concurrently
            w1a = xw.tile([128, 27, 128], fp16)
            nc.sync.dma_start(w1a[:, :, :], w1a_d.ap())
            nc.sync.dma_start(xA[:, 0, 0:6, :, :], xa_d.ap()[:, 0, 0:6 * 192])
            nc.sync.dma_start(xB[:, 0, 0:6, :, :], xb_d.ap()[:, 0, 0:6 * 192])
            w1b = xw.tile([128, 27, 128], fp16)
            nc.sync.dma_start(w1b[:, :, :], w1b_d.ap())
            nc.sync.dma_start(xA[:, 1, 0:6, :, :], xa_d.ap()[:, 1, 0:6 * 192])
            nc.sync.dma_start(xB[:, 1, 0:6, :, :], xb_d.ap()[:, 1, 0:6 * 192])
            aux = xw.tile([128, 4], fp32)
            nc.sync.dma_start(aux[:, :], aux_d.ap())
            nc.sync.dma_start(xA[:, 0, 6:16, :, :], xa_d.ap()[:, 0, 6 * 192:XPL])
            nc.sync.dma_start(xB[:, 0, 6:16, :, :], xb_d.ap()[:, 0, 6 * 192:XPL])
            nc.sync.dma_start(xA[:, 1, 6:16, :, :], xa_d.ap()[:, 1, 6 * 192:XPL])
            nc.sync.dma_start(xB[:, 1, 6:16, :, :], xb_d.ap()[:, 1, 6 * 192:XPL])
            nc.sync.dma_start(xA[:, 2:4, :, :, :], xa_d.ap()[:, 2:4, :])
            nc.sync.dma_start(xB[:, 2:4, :, :, :], xb_d.ap()[:, 2:4, :])
            b1 = aux[:, 0:1]
            mt = aux[:, 1:2]
            mb = aux[:, 2:3]
            b2 = aux[0:64, 3:4]

            w2t = xw.tile([128, 4, 27, 64], fp16)
            nc.sync.dma_start(w2t[:, :, :, :], w2_d.ap())

            hT = xw.tile([128, T, 4, D, HR, KW], fp16)

            # prime the h_e/h_o staging buffers so their pad cols stay zero
            for _ in range(2):
                he = stp.tile([128, 3, HR, 18], fp16)
                nc.vector.memset(he[:, :, :, :], 0.0)
                ho = stp.tile([128, 3, HR, 18], fp16)
                nc.vector.memset(ho[:, :, :, :], 0.0)

            # ---- conv1 (winograd-W components, K=64 row-split co-stream) ----
            for t in range(T):
                kts = _t_taps(t)
                for dlo, nd in RUNS1:
                    kus = _ku_valid(dlo, nd)
                    n = nd * HR * KW
                    ps0 = pp.tile([128, 512], fp32)
                    ps1 = pp.tile([128, 512], fp32)
                    ps2 = pp.tile([128, 512], fp32)
                    ps3 = pp.tile([128, 512], fp32)
                    pss = (ps0, ps1, ps2, ps3)
                    taps = [(kt, ku, kv) for kt in kts for ku in kus
                            for kv in range(3)]
                    for i, (kt, ku, kv) in enumerate(taps):
                        tp = t + kt - 1
                        dp = dlo + ku - 1
                        g = _g27(kt, ku, kv)
                        st_f = (i == 0)
                        sp_f = (i == len(taps) - 1)
                        for c, (xt, wt) in enumerate(((xA, w1a), (xA, w1a),
                                                      (xB, w1b), (xB, w1b))):
                            p0 = 64 * (c % 2)
                            nc.tensor.matmul(
                                pss[c][:, 0:n], wt[p0:p0 + 64, g, :],
                                xt[p0:p0 + 64, tp, dp:dp + nd, kv:kv + HR, :],
                                start=st_f, stop=sp_f, tile_position=(p0, 0))
                    # inverse: h_even = ReLU(m0+m1+m2+b1), h_odd = ReLU(m1-m2-m3+b1)
                    # (read ps0/ps1 earliest: the next run's first chains
                    #  reuse those banks)
                    cst = stp.tile([128, 512], fp16)
                    nc.scalar.activation(cst[:, 0:n], ps1[:, 0:n], IDENT)
                    ust = stp.tile([128, 512], fp16)
                    nc.vector.tensor_add(ust[:, 0:n], cst[:, 0:n], ps0[:, 0:n])
                    nc.vector.tensor_add(ust[:, 0:n], ust[:, 0:n], ps2[:, 0:n])
                    vst = stp.tile([128, 512], fp16)
                    nc.vector.tensor_sub(vst[:, 0:n], cst[:, 0:n], ps2[:, 0:n])
                    nc.vector.tensor_sub(vst[:, 0:n], vst[:, 0:n], ps3[:, 0:n])
                    he = stp.tile([128, 3, HR, 18], fp16)
                    nc.scalar.activation(he[:, 0:nd, :, 1:17],
                                         ust[:, 0:n], RELU, bias=b1[:, 0:1])
                    ho = stp.tile([128, 3, HR, 18], fp16)
                    nc.scalar.activation(ho[:, 0:nd, :, 1:17],
                                         vst[:, 0:n], RELU, bias=b1[:, 0:1])
                    # zero out-of-image h halo rows (edge cores only)
                    nc.vector.tensor_scalar_mul(
                        he[:, 0:nd, 0, 1:17], he[:, 0:nd, 0, 1:17], mt[:, 0:1])
                    nc.vector.tensor_scalar_mul(
                        ho[:, 0:nd, 0, 1:17], ho[:, 0:nd, 0, 1:17], mt[:, 0:1])
                    nc.vector.tensor_scalar_mul(
                        he[:, 0:nd, HR - 1, 1:17], he[:, 0:nd, HR - 1, 1:17],
                        mb[:, 0:1])
                    nc.vector.tensor_scalar_mul(
                        ho[:, 0:nd, HR - 1, 1:17], ho[:, 0:nd, HR - 1, 1:17],
                        mb[:, 0:1])
                    # h~ components: h~0=ho[k-1]-ho[k], h~1=he+ho, h~2=ho-he,
                    #                h~3=he[k]-he[k+1]
                    nc.vector.tensor_sub(hT[:, t, 0, dlo:dlo + nd, :, :],
                                         ho[:, 0:nd, :, 0:16],
                                         ho[:, 0:nd, :, 1:17])
                    nc.vector.tensor_add(hT[:, t, 1, dlo:dlo + nd, :, :],
                                         he[:, 0:nd, :, 1:17],
                                         ho[:, 0:nd, :, 1:17])
                    nc.vector.tensor_sub(hT[:, t, 2, dlo:dlo + nd, :, :],
                                         ho[:, 0:nd, :, 1:17],
                                         he[:, 0:nd, :, 1:17])
                    nc.vector.tensor_sub(hT[:, t, 3, dlo:dlo + nd, :, :],
                                         he[:, 0:nd, :, 1:17],
                                         he[:, 0:nd, :, 2:18])

            # ---- conv2 (winograd-W components, K=128 col-split co-stream) ----
            RUNS2_LAST = [(1, 4), (5, 4), (9, 4), (13, 2), (0, 1), (15, 1)]
            c2run = 0
            for t in range(T):
                kts = _t_taps(t)
                for dlo, nd in (RUNS2 if t < T - 1 else RUNS2_LAST):
                    kus = _ku_valid(dlo, nd)
                    n = nd * SH * KW
                    if c2run % 2 == 0:
                        ps0 = pp.tile([128, 512], fp32)
                        ps1 = pp.tile([128, 512], fp32)
                    else:
                        ps2 = pp.tile([128, 512], fp32)
                        ps3 = pp.tile([128, 512], fp32)
                        ps0, ps1 = ps2, ps3
                    c2run += 1
                    taps = [(kt, ku, kv) for kt in kts for ku in kus
                            for kv in range(3)]
                    for i, (kt, ku, kv) in enumerate(taps):
                        tp = t + kt - 1
                        dp = dlo + ku - 1
                        g = _g27(kt, ku, kv)
                        st_f = (i == 0)
                        sp_f = (i == len(taps) - 1)
                        for c in range(4):
                            ps = ps0 if c < 2 else ps1
                            base = 64 * (c % 2)
                            nc.tensor.matmul(
                                ps[base:base + 64, 0:n], w2t[:, c, g, :],
                                hT[:, tp, c, dp:dp + nd, kv:kv + SH, :],
                                start=st_f, stop=sp_f, tile_position=(0, base))
                    # inverse: y_even = m0+m1+m2+b2, y_odd = m1-m2-m3+b2
                    cst = stp.tile([64, 512], fp16)
                    nc.scalar.activation(cst[:, 0:n], ps0[64:128, 0:n], IDENT)
                    ust = stp.tile([128, 512], fp16)
                    nc.vector.tensor_add(ust[0:64, 0:n], cst[:, 0:n],
                                         ps0[0:64, 0:n])
                    nc.vector.tensor_add(ust[0:64, 0:n], ust[0:64, 0:n],
                                         ps1[0:64, 0:n])
                    vst = stp.tile([128, 512], fp16)
                    nc.vector.tensor_sub(vst[0:64, 0:n], cst[:, 0:n],
                                         ps1[0:64, 0:n])
                    nc.vector.tensor_sub(vst[0:64, 0:n], vst[0:64, 0:n],
                                         ps1[64:128, 0:n])
                    ye2 = stp.tile([64, 512], fp16)
                    nc.scalar.activation(ye2[:, 0:n], ust[0:64, 0:n], IDENT,
                                         bias=b2[:, 0:1])
                    yo2 = stp.tile([64, 512], fp16)
                    nc.scalar.activation(yo2[:, 0:n], vst[0:64, 0:n], IDENT,
                                         bias=b2[:, 0:1])
                    base = dlo * SH * KW
                    nc.sync.dma_start(y_d.ap()[:, t, 0, base:base + n],
                                      ye2[:, 0:n])
                    nc.sync.dma_start(y_d.ap()[:, t, 1, base:base + n],
                                      yo2[:, 0:n])
    nc.compile()
    return nc


def kernel(x, w1, b1, w2, b2):
    from concourse.bass_utils import run_bass_kernel_spmd

    hostd = _make_host_arrays(x, w1, b1, w2, b2)
    if "nc" not in _cache:
        _cache["nc"] = _build_module()
    nc = _cache["nc"]

    in_maps = []
    for core in range(NCORES):
        in_maps.append({
            "xa": hostd["XA"][core], "xb": hostd["XB"][core],
            "aux": hostd["AUX"][core],
            "w1a": hostd["W1A"], "w1b": hostd["W1B"], "w2": hostd["W2"],
        })
    res = run_bass_kernel_spmd(nc, in_maps, core_ids=list(range(NCORES)))

    y = np.zeros((B, C_OUT, T, D, H, W), np.float32)
    for core in range(NCORES):
        b, j = divmod(core, NJ)
        yc = res.results[core]["y"].reshape(C_OUT, T, 2, D, SH, KW)
        ys = y[b, :, :, :, SH * j:SH * (j + 1), :]
        ys[..., 0::2] = yc[:, :, 0]
        ys[..., 1::2] = yc[:, :, 1]
    return y
